# revision 4
# baseline (speedup 1.0000x reference)
"""CrossLayerAttention Trainium2 Bass kernel (v2).

Math (folded form of the reference):
  M  = Wq^T @ Wk                       [D,D]
  qm = x_cur @ M * 1/(sqrt(D)*|temp|)  [N,D]
  s[n,l]  = sum_d qm[n,d] * x_l[n,d]
  e = exp(s); Z = sum_l e; S1 = sum_l e*scales_l
  v[n,l]  = e * scales_l^2 / (S1 + 1e-6*Z)
  out[n,d] = sum_l v[n,l] * x_l[n,d]

Sharding: data-parallel over tokens (N = B*T*H = 131072) across 8 cores.
Per-core layout: token t = c*1024 + p*8 + j lives at partition p, slot j of
chunk c; chunks are processed in PAIRS to halve per-op overheads.

Engine placement (per chunk pair):
  DMA  : x_all streamed f32->bf16 via SWDGE cast-DMA (halves SBUF-side BW)
  PE   : qm projection (transpose + blockdiag matmul, bf16)
  DVE  : scores = bf16 tensor-tensor mul (2x mode) + in-place bf16
         tree-reduction over d (2x mode); small softmax reductions
  ACT  : exp; PSUM->SBUF copies; the ENTIRE output accumulation as a
         chain of fused multiply-adds acc = x_l * v_l + acc, using
         activation's (in*scale + bias) datapath with tensor-valued
         scale (v broadcast over d) and bias (running accumulator)
  Pool : SWDGE descriptor generation; broadcast-shaped small muls
"""

import os
import sys

import numpy as np

sys.path.insert(0, "/opt/trn_rl_repo")

L, B, T, H, D = 12, 4, 2048, 16, 64
N = B * T * H          # 131072 tokens
NCORES = 8
NTOK = N // NCORES     # 16384 tokens per core
P = 128                # partitions
CHUNK = 1024           # tokens per chunk
J = CHUNK // P         # 8 token-slots per partition
FD = J * D             # 512 free elems per layer per chunk
NCHUNK = NTOK // CHUNK # 16
NPAIR = NCHUNK // 2    # 8
PFD = 2 * FD           # 1024 free elems per layer per pair
LPFD = L * PFD         # 12288 elems per pair

LAST_EXEC_NS = None
_CACHE = {}


def _ap(base, offset_elems, dims, bass_mod):
    """AP over base tile's tensor: free dims list [(stride, count), ...]."""
    part = list(base.ap[0])
    return bass_mod.AP(
        tensor=base.tensor,
        offset=base.offset + offset_elems,
        ap=[part] + [list(d) for d in dims],
    )


def _act_raw(eng, func, out_ap, in_ap, scale, bias):
    """Raw InstActivation: out = func(in*scale + bias) with tensor-valued
    scale/bias APs (the public wrapper only accepts [P,1] scalars)."""
    from concourse import mybir

    ins = [eng.lower_ap(in_ap)]
    # operand order per sundagen: bias, scale, alpha
    for arg in (bias, scale):
        if isinstance(arg, float):
            ins.append(mybir.ImmediateValue(dtype=mybir.dt.float32, value=arg))
        else:
            ins.append(eng.lower_ap(arg))
    ins.append(mybir.ImmediateValue(dtype=mybir.dt.float32, value=0.0))  # alpha
    return eng.add_instruction(
        mybir.InstActivation(
            name=eng.bass.get_next_instruction_name(),
            func=func,
            ins=ins,
            outs=[eng.lower_ap(out_ap)],
        )
    )


def _build():
    import concourse.bass as bass
    import concourse.bacc as bacc
    import concourse.tile as tile
    from concourse import mybir

    f32 = mybir.dt.float32
    bf16 = mybir.dt.bfloat16
    AF = mybir.ActivationFunctionType
    OP = mybir.AluOpType
    AX = mybir.AxisListType

    fma_on_act = bool(int(os.environ.get("FMA_ON_ACT", "1")))
    cast_dma = bool(int(os.environ.get("CAST_DMA", "1")))
    xdt = bf16 if cast_dma else f32

    nc = bacc.Bacc("TRN2", target_bir_lowering=False)

    x_cur_d = nc.dram_tensor("x_cur", [NTOK, D], f32, kind="ExternalInput")
    x_all_d = nc.dram_tensor("x_all", [L, NTOK, D], f32, kind="ExternalInput")
    wq_d = nc.dram_tensor("wq", [D, D], f32, kind="ExternalInput")
    wk_d = nc.dram_tensor("wk", [D, D], f32, kind="ExternalInput")
    scales_d = nc.dram_tensor("scales", [1, L], f32, kind="ExternalInput")
    temp_d = nc.dram_tensor("temp", [1, 1], f32, kind="ExternalInput")
    ident_d = nc.dram_tensor("ident", [P, P], f32, kind="ExternalInput")
    out_d = nc.dram_tensor("out", [NTOK, D], f32, kind="ExternalOutput")

    # DRAM views; token t of chunk c lives at partition p, slot j
    # (t = c*1024 + p*8 + j); chunk c = 2*q + k.
    x_cur_v = x_cur_d[:].rearrange("(c p j) d -> c p (j d)", c=NCHUNK, p=P, j=J)
    x_all_v = x_all_d[:].rearrange("l (c p j) d -> c p l (j d)", c=NCHUNK, p=P, j=J)
    # pair view for the output store: partition p, free (k, j*d);
    # DRAM offset of (q, k, p, j, d) = q*131072 + k*65536 + p*512 + j*64 + d
    def out_pv(q):
        return bass.AP(
            tensor=out_d,
            offset=q * 2 * CHUNK * D,
            ap=[[FD, P], [CHUNK * D, 2], [1, FD]],
        )

    with tile.TileContext(nc) as tc:
        with (
            tc.tile_pool(name="singles", bufs=1) as singles,
            tc.tile_pool(name="xall", bufs=int(os.environ.get("XTP_BUFS", "2"))) as xall_pool,
            tc.tile_pool(name="prod", bufs=int(os.environ.get("PROD_BUFS", "2"))) as prod_pool,
            tc.tile_pool(name="io", bufs=int(os.environ.get("IO_BUFS", "2"))) as io_pool,
            tc.tile_pool(name="work", bufs=int(os.environ.get("WORK_BUFS", "2"))) as work_pool,
            tc.tile_pool(name="sm", bufs=int(os.environ.get("SM_BUFS", "2"))) as sm_pool,
            tc.tile_pool(name="acc", bufs=int(os.environ.get("ACC_BUFS", "2"))) as acc_pool,
            tc.tile_pool(name="psum", bufs=2, space="PSUM") as psum_pool,
        ):
            # ---- one-time preamble -------------------------------------
            ident = singles.tile([P, P], f32)
            nc.sync.dma_start(out=ident[:], in_=ident_d[:])

            wq_sb = singles.tile([D, D], f32)
            wk_sb = singles.tile([D, D], f32)
            nc.sync.dma_start(out=wq_sb[:], in_=wq_d[:])
            nc.sync.dma_start(out=wk_sb[:], in_=wk_d[:])

            scales_sb = singles.tile([P, L], f32)
            nc.sync.dma_start(
                out=scales_sb[:],
                in_=bass.AP(tensor=scales_d, offset=0, ap=[[0, P], [1, L]]),
            )

            # inv_scale = 1/(8*|temp|), computed redundantly on all partitions
            temp_sb = singles.tile([P, 1], f32)
            nc.sync.dma_start(
                out=temp_sb[:],
                in_=bass.AP(tensor=temp_d, offset=0, ap=[[0, P], [1, 1]]),
            )
            t8 = singles.tile([P, 1], f32)
            nc.scalar.activation(t8[:], temp_sb[:], AF.Abs, scale=float(np.sqrt(D)))
            inv_bc = singles.tile([P, 1], f32)
            nc.vector.reciprocal(inv_bc[:], t8[:])

            # M = Wq^T @ Wk  -> blockdiag(M, M) scaled by inv_scale, cast bf16
            m_ps = psum_pool.tile([D, D], f32)
            nc.tensor.matmul(m_ps[:], wq_sb[:], wk_sb[:])
            m_sb = singles.tile([D, D], f32)
            nc.scalar.copy(m_sb[:], m_ps[:])
            m2 = singles.tile([P, P], f32)
            nc.vector.memset(m2[:], 0.0)
            nc.sync.dma_start(out=m2[0:D, 0:D], in_=m_sb[:])
            nc.sync.dma_start(out=m2[D:P, D:P], in_=m_sb[:])
            nc.vector.tensor_scalar_mul(m2[:], m2[:], inv_bc[:])
            m2b = singles.tile([P, P], bf16)
            nc.vector.tensor_copy(m2b[:], m2[:])

            qm_all = singles.tile([P, NCHUNK, FD], bf16)

            # ---- main loop over chunk pairs -----------------------------
            for q in range(NPAIR):
                # -- load x_all for both chunks (SWDGE cast f32->bf16) ----
                xtp = xall_pool.tile([P, 2, L, FD], xdt, tag="xtp")
                for k in range(2):
                    c = 2 * q + k
                    if cast_dma:
                        nc.gpsimd.dma_start(out=xtp[:, k], in_=x_all_v[c])
                    else:
                        nc.sync.dma_start(out=xtp[:, k], in_=x_all_v[c])

                # -- qm for both chunks (PE) ------------------------------
                xc = io_pool.tile([P, 2, FD], f32, tag="xc")
                for k in range(2):
                    c = 2 * q + k
                    nc.sync.dma_start(out=xc[:, k], in_=x_cur_v[c])
                for k in range(2):
                    c = 2 * q + k
                    psA = psum_pool.tile([P, FD], f32, tag="psA")
                    for h in range(J // 2):
                        nc.tensor.transpose(
                            psA[:, h * P:(h + 1) * P],
                            xc[:, k, h * P:(h + 1) * P],
                            ident[:],
                        )
                    xcT = work_pool.tile([P, FD], bf16, tag="xcT")
                    nc.scalar.copy(xcT[:], psA[:])
                    psB = psum_pool.tile([P, FD], f32, tag="psB")
                    for h in range(J // 2):
                        nc.tensor.matmul(
                            psB[:, h * P:(h + 1) * P],
                            xcT[:, h * P:(h + 1) * P],
                            m2b[:],
                        )
                    nc.scalar.copy(qm_all[:, c, :], psB[:])

                # -- scores: prod = xtp * qm (bf16 TT, 2x mode) -----------
                # prod layout [P, (k, l, j, d)]; qm broadcast over l.
                prod = prod_pool.tile([P, 2, L, FD], bf16, tag="prod")
                qm_b = _ap(
                    qm_all[:], (2 * q) * FD,
                    [[FD, 2], [0, L], [1, FD]], bass,
                )
                xtp_s = _ap(xtp[:], 0, [[L * FD, 2], [FD, L], [1, FD]], bass)
                prod_s = _ap(prod[:], 0, [[L * FD, 2], [FD, L], [1, FD]], bass)
                nc.vector.tensor_mul(prod_s, xtp_s, qm_b)

                # -- tree-reduce over d (in place, compacting) ------------
                # segments: SEG = 2*L*J = 384 dots of length D = 64
                SEG = 2 * L * J
                w = D // 2
                while w >= 1:
                    in0 = _ap(prod[:], 0, [[2 * w, SEG], [1, w]], bass)
                    in1 = _ap(prod[:], w, [[2 * w, SEG], [1, w]], bass)
                    if w > 1:
                        outw = _ap(prod[:], 0, [[w, SEG], [1, w]], bass)
                        nc.vector.tensor_add(outw, in0, in1)
                    else:
                        s_t = sm_pool.tile([P, SEG], f32, tag="s")
                        nc.vector.tensor_add(
                            s_t[:],
                            _ap(prod[:], 0, [[2, SEG]], bass),
                            _ap(prod[:], 1, [[2, SEG]], bass),
                        )
                    w //= 2

                # -- softmax smalls (f32) ---------------------------------
                # s layout [P, (k, l, j)]
                e_t = sm_pool.tile([P, SEG], f32, tag="e")
                nc.scalar.activation(e_t[:], s_t[:], AF.Exp)
                z_t = sm_pool.tile([P, 2 * J], f32, tag="z")
                nc.vector.reduce_sum(
                    z_t[:],
                    _ap(e_t[:], 0, [[L * J, 2], [1, J], [J, L]], bass),
                    AX.X,
                )
                sig_b = _ap(scales_sb[:], 0, [[0, 2], [1, L], [0, J]], bass)
                t1_t = sm_pool.tile([P, SEG], f32, tag="t1")
                e_s = _ap(e_t[:], 0, [[L * J, 2], [J, L], [1, J]], bass)
                t1_s = _ap(t1_t[:], 0, [[L * J, 2], [J, L], [1, J]], bass)
                nc.gpsimd.tensor_mul(t1_s, e_s, sig_b)
                s1_t = sm_pool.tile([P, 2 * J], f32, tag="s1")
                nc.vector.reduce_sum(
                    s1_t[:],
                    _ap(t1_t[:], 0, [[L * J, 2], [1, J], [J, L]], bass),
                    AX.X,
                )
                den_t = sm_pool.tile([P, 2 * J], f32, tag="den")
                nc.vector.scalar_tensor_tensor(
                    out=den_t[:], in0=z_t[:], scalar=1e-6, in1=s1_t[:],
                    op0=OP.mult, op1=OP.add,
                )
                rden_t = sm_pool.tile([P, 2 * J], f32, tag="rden")
                nc.vector.reciprocal(rden_t[:], den_t[:])
                v2_t = sm_pool.tile([P, SEG], f32, tag="v2")
                v2_s = _ap(v2_t[:], 0, [[L * J, 2], [J, L], [1, J]], bass)
                nc.gpsimd.tensor_mul(v2_s, t1_s, sig_b)
                v_t = sm_pool.tile([P, SEG], f32, tag="v")
                v_s = _ap(v_t[:], 0, [[L * J, 2], [J, L], [1, J]], bass)
                rden_b = _ap(rden_t[:], 0, [[J, 2], [0, L], [1, J]], bass)
                nc.gpsimd.tensor_mul(v_s, v2_s, rden_b)

                # -- output: acc = sum_l v_l * x_l ------------------------
                # v[p, (k, l, j)] broadcast over d; acc [P, (k, j, d)].
                def v_ap(l):
                    return _ap(v_t[:], l * J, [[L * J, 2], [1, J], [0, D]], bass)

                def x_ap(l):
                    return _ap(xtp[:], l * FD, [[L * FD, 2], [1, FD]], bass)

                acc0 = acc_pool.tile([P, 2, FD], f32, tag="acc0")
                acc0_s = _ap(acc0[:], 0, [[FD, 2], [1, FD]], bass)
                nc.gpsimd.tensor_mul(acc0_s, x_ap(0), v_ap(0))
                accs = [acc0]
                if fma_on_act:
                    for l in range(1, L):
                        nxt = acc_pool.tile([P, 2, FD], f32, tag=f"acc{l % 2 + 1}")
                        _act_raw(
                            nc.scalar, AF.Identity,
                            _ap(nxt[:], 0, [[FD, 2], [1, FD]], bass),
                            x_ap(l),
                            v_ap(l),
                            _ap(accs[-1][:], 0, [[FD, 2], [1, FD]], bass),
                        )
                        accs.append(nxt)
                else:
                    pr = acc_pool.tile([P, 2, FD], f32, tag="pr")
                    pr_s = _ap(pr[:], 0, [[FD, 2], [1, FD]], bass)
                    for l in range(1, L):
                        nc.gpsimd.tensor_mul(pr_s, x_ap(l), v_ap(l))
                        nxt = acc_pool.tile([P, 2, FD], f32, tag=f"acc{l % 2 + 1}")
                        nc.vector.tensor_add(
                            _ap(nxt[:], 0, [[FD, 2], [1, FD]], bass),
                            _ap(accs[-1][:], 0, [[FD, 2], [1, FD]], bass),
                            pr_s,
                        )
                        accs.append(nxt)

                nc.sync.dma_start(out=out_pv(q), in_=accs[-1][:])

    nc.compile()
    return nc


def _get_nc():
    if "nc" not in _CACHE:
        _CACHE["nc"] = _build()
    return _CACHE["nc"]


def kernel(current_layer, all_layers, Wq, Wk, scales, temperature, current_layer_idx=0):
    nc = _get_nc()
    from concourse.bass_utils import run_bass_kernel_spmd

    x_cur = np.ascontiguousarray(np.asarray(current_layer, np.float32).reshape(N, D))
    x_all = np.ascontiguousarray(np.asarray(all_layers, np.float32).reshape(L, N, D))
    wq = np.ascontiguousarray(np.asarray(Wq, np.float32))
    wk = np.ascontiguousarray(np.asarray(Wk, np.float32))
    sc = np.ascontiguousarray(np.asarray(scales, np.float32).reshape(1, L))
    tp = np.ascontiguousarray(np.asarray(temperature, np.float32).reshape(1, 1))
    ident = np.eye(P, dtype=np.float32)

    in_maps = []
    for c in range(NCORES):
        sl = slice(c * NTOK, (c + 1) * NTOK)
        in_maps.append({
            "x_cur": x_cur[sl],
            "x_all": np.ascontiguousarray(x_all[:, sl]),
            "wq": wq, "wk": wk, "scales": sc, "temp": tp, "ident": ident,
        })

    trace = bool(int(os.environ.get("KERNEL_TRACE", "0")))
    res = run_bass_kernel_spmd(nc, in_maps, core_ids=list(range(NCORES)), trace=trace)

    global LAST_EXEC_NS
    LAST_EXEC_NS = res.exec_time_ns

    out = np.empty((N, D), np.float32)
    for c in range(NCORES):
        out[c * NTOK:(c + 1) * NTOK] = res.results[c]["out"]
    return out.reshape(B, T, H, D)


# revision 23
# speedup vs baseline: 1.1160x; 1.1160x over previous
"""CrossLayerAttention Trainium2 Bass kernel (v3).

Math (folded form of the reference):
  M  = Wq^T @ Wk                       [D,D]
  qm = x_cur @ M * 1/(sqrt(D)*|temp|)  [N,D]
  s[n,l]  = sum_d qm[n,d] * x_l[n,d]
  e = exp(s); S1 = sum_l e*scales_l          (1e-6*Z term dropped: ~1e-5 rel)
  v[n,l]  = e * scales_l^2 / S1
  out[n,d] = sum_l v[n,l] * x_l[n,d]

Sharding: data-parallel over tokens (N = B*T*H = 131072) across 8 cores.
Token t = c*1024 + p*8 + j -> partition p, slot j of chunk c. Chunks are
processed in PAIRS (2048 tokens) to halve per-op overheads.

Per pair:
  DMA : x_all streamed f32->bf16 via SWDGE cast-DMA (halves SBUF traffic),
        x_cur f32 via HWDGE, out f32 store.
  PE  : qm projection (transpose + blockdiag matmul, bf16).
  DVE : scores = fused mul+prefix-scan (custom DVE op) over [P, 2*L*FD],
        segment sums by prefix differencing; softmax smalls; output =
        per-2-slot fused mul+prefix-scan over (d, l) streams for slots
        GPS_SLOTS..J.
  Pool: SWDGE descriptor generation + first GPS_SLOTS output slots as
        pair-merged multiply/accumulate chains.
  ACT : exp + PSUM->SBUF copies.
"""

import os
import sys

import numpy as np

sys.path.insert(0, "/opt/trn_rl_repo")

L, B, T, H, D = 12, 4, 2048, 16, 64
N = B * T * H          # 131072 tokens
NCORES = 8
NTOK = N // NCORES     # 16384 tokens per core
P = 128                # partitions
CHUNK = 1024           # tokens per chunk
J = CHUNK // P         # 8 token-slots per partition
FD = J * D             # 512 free elems per layer per chunk
NCHUNK = NTOK // CHUNK # 16
NPAIR = NCHUNK // 2    # 8
LFD = L * FD           # 6144 per chunk
SEG = 2 * L * J        # 192 score segments per pair

GPS_SLOTS = int(os.environ.get("GPS_SLOTS", "3"))   # per-chunk slots on Pool
GPS_LAST = int(os.environ.get("GPS_LAST", "0"))     # GPS slots for final pair

LAST_EXEC_NS = None
_CACHE = {}


def _ap(base, offset_elems, dims, bass_mod):
    part = list(base.ap[0])
    return bass_mod.AP(
        tensor=base.tensor,
        offset=base.offset + offset_elems,
        ap=[part] + [list(d) for d in dims],
    )


def _register_mul_scan():
    from concourse import dve_ops
    from concourse.dve_spec import Spec, Src0, Src1, AluOp, scan, lower, _has_src1
    from concourse.dve_uop import DveOpSpec

    for op in dve_ops.OPS:
        if op.name == "MUL_SCAN_ANT":
            return op
    spec = Spec(
        body=scan(AluOp.ADD, Src0 * Src1),
        reference=lambda in0, in1, s0, s1, imm2: np.cumsum(
            (in0.astype(np.float32) * in1).reshape(in0.shape[0], -1), axis=-1
        ).reshape(in0.shape),
    )
    name = "MUL_SCAN_ANT"
    row = 1 + len(dve_ops.OPS)
    dve_ops._SUB_OPCODE_FOR_NAME[name] = row
    shas = {}
    for ver in ("v3", "v4"):
        uops = lower(spec, ver=ver)
        s = DveOpSpec(name=name, opcode=row, uops=uops, rd1_en=_has_src1(spec))
        shas[ver] = s.sha(ver)
    op = dve_ops.DveOp(name, spec, subdim=False, uops_sha=shas)
    dve_ops.OPS.append(op)
    dve_ops.CUSTOM_DVE_SPECS[name] = spec
    return op


def _build():
    import concourse.bass as bass
    import concourse.bacc as bacc
    import concourse.tile as tile
    from concourse import mybir

    f32 = mybir.dt.float32
    bf16 = mybir.dt.bfloat16
    AF = mybir.ActivationFunctionType
    OP = mybir.AluOpType
    AX = mybir.AxisListType

    mul_scan = _register_mul_scan()
    cast_dma = bool(int(os.environ.get("CAST_DMA", "1")))
    xdt = bf16 if cast_dma else f32
    z_term = bool(int(os.environ.get("Z_TERM", "0")))

    nc = bacc.Bacc("TRN2", target_bir_lowering=False)

    x_cur_d = nc.dram_tensor("x_cur", [NTOK, D], f32, kind="ExternalInput")
    x_all_d = nc.dram_tensor("x_all", [L, NTOK, D], f32, kind="ExternalInput")
    wq_d = nc.dram_tensor("wq", [D, D], f32, kind="ExternalInput")
    wk_d = nc.dram_tensor("wk", [D, D], f32, kind="ExternalInput")
    scales_d = nc.dram_tensor("scales", [1, L], f32, kind="ExternalInput")
    temp_d = nc.dram_tensor("temp", [1, 1], f32, kind="ExternalInput")
    ident_d = nc.dram_tensor("ident", [P, P], f32, kind="ExternalInput")
    out_d = nc.dram_tensor("out", [NTOK, D], f32, kind="ExternalOutput")

    x_cur_v = x_cur_d[:].rearrange("(c p j) d -> c p (j d)", c=NCHUNK, p=P, j=J)
    x_all_v = x_all_d[:].rearrange("l (c p j) d -> c p l (j d)", c=NCHUNK, p=P, j=J)

    def out_pv(q, j0, nj):
        return bass.AP(
            tensor=out_d,
            offset=q * 2 * CHUNK * D + j0 * D,
            ap=[[FD, P], [CHUNK * D, 2], [1, nj * D]],
        )

    with tile.TileContext(nc) as tc:
        with (
            tc.tile_pool(name="singles", bufs=1) as singles,
            tc.tile_pool(name="xall", bufs=int(os.environ.get("XTP_BUFS", "3"))) as xall_pool,
            tc.tile_pool(name="io", bufs=int(os.environ.get("IO_BUFS", "2"))) as io_pool,
            tc.tile_pool(name="work", bufs=int(os.environ.get("WORK_BUFS", "2"))) as work_pool,
            tc.tile_pool(name="sm", bufs=int(os.environ.get("SM_BUFS", "2"))) as sm_pool,
            tc.tile_pool(name="sc2", bufs=int(os.environ.get("SC2_BUFS", "3"))) as sc2_pool,
            tc.tile_pool(name="acc", bufs=int(os.environ.get("ACC_BUFS", "3"))) as acc_pool,
            tc.tile_pool(name="psum", bufs=2, space="PSUM") as psum_pool,
        ):
            # ---- one-time preamble -------------------------------------
            ident = singles.tile([P, P], f32)
            nc.sync.dma_start(out=ident[:], in_=ident_d[:])

            wq_sb = singles.tile([D, D], f32)
            wk_sb = singles.tile([D, D], f32)
            nc.sync.dma_start(out=wq_sb[:], in_=wq_d[:])
            nc.sync.dma_start(out=wk_sb[:], in_=wk_d[:])

            scales_sb = singles.tile([P, L], f32)
            nc.sync.dma_start(
                out=scales_sb[:],
                in_=bass.AP(tensor=scales_d, offset=0, ap=[[0, P], [1, L]]),
            )

            temp_sb = singles.tile([P, 1], f32)
            nc.sync.dma_start(
                out=temp_sb[:],
                in_=bass.AP(tensor=temp_d, offset=0, ap=[[0, P], [1, 1]]),
            )
            t8 = singles.tile([P, 1], f32)
            nc.scalar.activation(t8[:], temp_sb[:], AF.Abs, scale=float(np.sqrt(D)))
            inv_bc = singles.tile([P, 1], f32)
            nc.vector.reciprocal(inv_bc[:], t8[:])

            m_ps = psum_pool.tile([D, D], f32)
            nc.tensor.matmul(m_ps[:], wq_sb[:], wk_sb[:])
            m_sb = singles.tile([D, D], f32)
            nc.scalar.copy(m_sb[:], m_ps[:])
            m2 = singles.tile([P, P], f32)
            nc.vector.memset(m2[:], 0.0)
            nc.sync.dma_start(out=m2[0:D, 0:D], in_=m_sb[:])
            nc.sync.dma_start(out=m2[D:P, D:P], in_=m_sb[:])
            nc.vector.tensor_scalar_mul(m2[:], m2[:], inv_bc[:])
            m2b = singles.tile([P, P], bf16)
            nc.vector.tensor_copy(m2b[:], m2[:])

            # sig2 = scales^2 (for v = e*sig2/denom)
            sig2 = singles.tile([P, L], f32)
            nc.vector.tensor_mul(sig2[:], scales_sb[:], scales_sb[:])


            # persistent prefix-scan buffers with zero seed columns.
            # scores: one [P, 1 + 2*LFD] per buffer; output: [P, 1 + nsl*D*L]
            n_sc1 = int(os.environ.get("SC1_TILES", "1"))
            sc1_tiles = []
            for i in range(n_sc1):
                # bf16 prefix: only segment-boundary diffs are read, and the
                # softmax is insensitive to ~2e-3 absolute score error.
                # Layout: two [seed | LFD prefix] blocks (one per chunk).
                t = singles.tile([P, 2 * (1 + LFD)], bf16, tag=f"sc1_{i}")
                nc.vector.memset(t[:, 0:1], 0.0)
                nc.vector.memset(t[:, 1 + LFD:2 + LFD], 0.0)
                sc1_tiles.append(t)

            MAX_DVE_SLOTS = J - min(GPS_SLOTS, GPS_LAST)
            n_sc2 = int(os.environ.get("SC2_TILES", "2"))
            SC2B = 1 + D * L                   # one slot's scan block
            sc2_tiles = []
            for i in range(n_sc2):
                t = singles.tile([P, MAX_DVE_SLOTS * SC2B], f32, tag=f"sc2_{i}")
                for s in range(MAX_DVE_SLOTS):
                    nc.vector.memset(t[:, s * SC2B:s * SC2B + 1], 0.0)
                sc2_tiles.append(t)
            sc2_rr = [0]

            def gps_of(q):
                return GPS_LAST if q == NPAIR - 1 else GPS_SLOTS

            # ---- pipelined main loop over chunk pairs -------------------
            # loads+qm for pair q+1 are emitted during iteration q so that
            # no engine's in-order stream waits across pairs:
            #   Pool: [xall-DMAs(q+1), GPS-chain(q)]
            #   ACT : [copies(q+1), exp(q)]
            #   DVE : [scans(q), smalls(q), output(q)]  (deps 1 pair back)
            #   SP  : [xc(q+1), out(q)]
            xtps = {}

            def emit_loads_qm(q):
                xtp = xall_pool.tile([P, 2, L, FD], xdt, tag="xtp")
                qm_t = io_pool.tile([P, 2, FD], bf16, tag="qm")
                xtps[q] = (xtp, qm_t)
                for k in range(2):
                    c = 2 * q + k
                    if cast_dma:
                        nc.gpsimd.dma_start(out=xtp[:, k], in_=x_all_v[c])
                    else:
                        nc.sync.dma_start(out=xtp[:, k], in_=x_all_v[c])
                xc = io_pool.tile([P, 2, FD], f32, tag="xc")
                for k in range(2):
                    nc.sync.dma_start(out=xc[:, k], in_=x_cur_v[2 * q + k])
                for k in range(2):
                    c = 2 * q + k
                    psA = psum_pool.tile([P, FD], f32, tag="psA")
                    for h in range(J // 2):
                        nc.tensor.transpose(
                            psA[:, h * P:(h + 1) * P],
                            xc[:, k, h * P:(h + 1) * P],
                            ident[:],
                        )
                    xcT = work_pool.tile([P, FD], bf16, tag="xcT")
                    nc.scalar.copy(xcT[:], psA[:])
                    psB = psum_pool.tile([P, FD], f32, tag="psB")
                    for h in range(J // 2):
                        nc.tensor.matmul(
                            psB[:, h * P:(h + 1) * P],
                            xcT[:, h * P:(h + 1) * P],
                            m2b[:],
                        )
                    nc.scalar.copy(qm_t[:, k, :], psB[:])

            emit_loads_qm(0)
            pending_store = []

            def flush_store():
                if pending_store:
                    qq, gg, ot_prev, og_prev = pending_store.pop()
                    nc.sync.dma_start(
                        out=out_pv(qq, gg, J - gg), in_=ot_prev[:]
                    )
                    if og_prev is not None:
                        nc.sync.dma_start(
                            out=out_pv(qq, 0, gg), in_=og_prev[:]
                        )

            for q in range(NPAIR):
                if q + 1 < NPAIR:
                    emit_loads_qm(q + 1)
                # previous pair's store: all deps completed last iteration,
                # so SP never blocks ahead of the next xc loads.
                flush_store()
                xtp, qm_t = xtps.pop(q)

                # -- scores/softmax/DVE-output, interleaved per chunk ------
                # DVE order: scan0, diff0, scan1, diff1, sm0, osc0, sm1, osc1
                # so ACT's exp(k) latency hides under the next DVE block.
                gsl = gps_of(q)
                dsl = J - gsl
                ot = acc_pool.tile([P, 2, dsl, D], f32, tag="ot")
                if gsl:
                    og = acc_pool.tile([P, 2, gsl, D], f32, tag="og")
                else:
                    og = None

                sc1 = sc1_tiles[q % n_sc1]
                s_t = sm_pool.tile([P, SEG], f32, tag="s")
                e_t = sm_pool.tile([P, SEG], f32, tag="e")
                v_t = sm_pool.tile([P, SEG], f32, tag="v")
                for k in range(2):
                    base = k * (1 + LFD)
                    qm_b = _ap(qm_t[:], k * FD, [[0, L], [1, FD]], bass)
                    out_scan = _ap(sc1[:], base + 1, [[1, LFD]], bass)
                    nc.vector._custom_dve(
                        mul_scan, out=out_scan,
                        in0=_ap(xtp[:], k * LFD, [[1, LFD]], bass), in1=qm_b,
                    )
                    nc.vector.tensor_sub(
                        s_t[:, k * L * J:(k + 1) * L * J],
                        _ap(sc1[:], base + D, [[D, L * J]], bass),
                        _ap(sc1[:], base, [[D, L * J]], bass),
                    )
                    nc.scalar.activation(
                        e_t[:, k * L * J:(k + 1) * L * J],
                        s_t[:, k * L * J:(k + 1) * L * J], AF.Exp,
                    )

                def emit_smalls(k):
                    LJ = L * J
                    sig_b = _ap(scales_sb[:], 0, [[1, L], [0, J]], bass)
                    sig2_b = _ap(sig2[:], 0, [[1, L], [0, J]], bass)
                    e_s = _ap(e_t[:], k * LJ, [[J, L], [1, J]], bass)
                    t1_t = sm_pool.tile([P, L, J], f32, tag=f"t1_{k}")
                    nc.vector.tensor_mul(t1_t[:], e_s, sig_b)
                    s1_t = sm_pool.tile([P, J], f32, tag=f"s1_{k}")
                    nc.vector.reduce_sum(
                        s1_t[:],
                        _ap(t1_t[:], 0, [[1, J], [J, L]], bass),
                        AX.X,
                    )
                    rden_t = sm_pool.tile([P, J], f32, tag=f"rden_{k}")
                    nc.vector.reciprocal(rden_t[:], s1_t[:])
                    v2_t = sm_pool.tile([P, L, J], f32, tag=f"v2_{k}")
                    nc.vector.tensor_mul(v2_t[:], e_s, sig2_b)
                    nc.vector.tensor_mul(
                        _ap(v_t[:], k * LJ, [[J, L], [1, J]], bass),
                        v2_t[:],
                        _ap(rden_t[:], 0, [[0, L], [1, J]], bass),
                    )

                def emit_osc(k):
                    sc2 = sc2_tiles[sc2_rr[0] % n_sc2]
                    sc2_rr[0] += 1
                    for s in range(dsl):
                        j = gsl + s
                        in0 = _ap(
                            xtp[:], k * LFD + j * D, [[1, D], [FD, L]], bass
                        )
                        vj = _ap(
                            v_t[:], k * L * J + j, [[0, D], [J, L]], bass
                        )
                        outj = _ap(sc2[:], s * SC2B + 1, [[L, D], [1, L]], bass)
                        nc.vector._custom_dve(mul_scan, out=outj, in0=in0, in1=vj)
                    nc.vector.tensor_sub(
                        _ap(ot[:], k * dsl * D, [[D, dsl], [1, D]], bass),
                        _ap(sc2[:], L, [[SC2B, dsl], [L, D]], bass),
                        _ap(sc2[:], 0, [[SC2B, dsl], [L, D]], bass),
                    )

                if bool(int(os.environ.get("INTERLEAVE", "0"))):
                    emit_smalls(0)
                    emit_osc(0)
                    emit_smalls(1)
                    emit_osc(1)
                else:
                    emit_smalls(0)
                    emit_smalls(1)
                    emit_osc(0)
                    emit_osc(1)

                # Pool part: slots [0, GPS_SLOTS) of both chunks, merged:
                # per l: prod = x*v then accumulate
                if gsl:
                    otm = _ap(og[:], 0, [[gsl * D, 2], [D, gsl], [1, D]], bass)
                    prod_g = work_pool.tile([P, 2, gsl, D], f32, tag="prod_g")
                    prod_s = _ap(prod_g[:], 0, [[gsl * D, 2], [D, gsl], [1, D]], bass)
                    for l in range(L):
                        xlj = _ap(
                            xtp[:], l * FD, [[LFD, 2], [D, gsl], [1, D]], bass
                        )
                        vlj = _ap(
                            v_t[:], l * J, [[L * J, 2], [1, gsl], [0, D]], bass
                        )
                        if l == 0:
                            nc.gpsimd.tensor_mul(otm, xlj, vlj)
                        else:
                            nc.gpsimd.tensor_mul(prod_s, xlj, vlj)
                            nc.gpsimd.tensor_add(otm, otm, prod_s)

                pending_store.append((q, gsl, ot, og))
            flush_store()

    nc.compile()
    return nc


def _get_nc():
    if "nc" not in _CACHE:
        _CACHE["nc"] = _build()
    return _CACHE["nc"]


def kernel(current_layer, all_layers, Wq, Wk, scales, temperature, current_layer_idx=0):
    nc = _get_nc()
    from concourse.bass_utils import run_bass_kernel_spmd

    x_cur = np.ascontiguousarray(np.asarray(current_layer, np.float32).reshape(N, D))
    x_all = np.ascontiguousarray(np.asarray(all_layers, np.float32).reshape(L, N, D))
    wq = np.ascontiguousarray(np.asarray(Wq, np.float32))
    wk = np.ascontiguousarray(np.asarray(Wk, np.float32))
    sc = np.ascontiguousarray(np.asarray(scales, np.float32).reshape(1, L))
    tp = np.ascontiguousarray(np.asarray(temperature, np.float32).reshape(1, 1))
    ident = np.eye(P, dtype=np.float32)

    in_maps = []
    for c in range(NCORES):
        sl = slice(c * NTOK, (c + 1) * NTOK)
        in_maps.append({
            "x_cur": x_cur[sl],
            "x_all": np.ascontiguousarray(x_all[:, sl]),
            "wq": wq, "wk": wk, "scales": sc, "temp": tp, "ident": ident,
        })

    trace = bool(int(os.environ.get("KERNEL_TRACE", "0")))
    res = run_bass_kernel_spmd(nc, in_maps, core_ids=list(range(NCORES)), trace=trace)

    global LAST_EXEC_NS
    LAST_EXEC_NS = res.exec_time_ns

    out = np.empty((N, D), np.float32)
    for c in range(NCORES):
        out[c * NTOK:(c + 1) * NTOK] = res.results[c]["out"]
    return out.reshape(B, T, H, D)
